# revision 10
# baseline (speedup 1.0000x reference)
"""MultiHeadCrossWindowAttention Bass/Tile kernel for 8 trn2 NeuronCores.

Strategy (data-parallel over batch, 1 batch item per core):
  - Fold Wq/Wk into G = Wq.T @ Wk / sqrt(hd)  (shared across heads):
      scores^T = x_w^T G^T x_w  per window  ->  u = G^T x (streaming matmul),
      s^T[ki,qi] = sum_d x[d,ki] u[d,qi]  (per-window matmul, lhsT = raw x).
  - Fold Wl @ Wv into H: y = (H skip_w) p_hat:
      w1 = (H skip_w)^T  via lhsT = skip_w, rhs = H^T,
      y[f,qi] = sum_ki w1[ki,f] p_hat[ki,qi]  (channel-major output).
  - softmax: p = exp(s^T) * ET (ET = exp(bias+mask) tables, multiplicative),
      Z broadcast to all partitions via all-ones [64,64] stationary matmul,
      y normalized during the PSUM->SBUF copy (tensor_mul with 1/Z).
  - Shifted-window roll (+-4) folded into HBM<->SBUF DMA access patterns.

Layout per core: 4 head-pairs x 16 window-rows x 2 halves = 128 supertiles,
each supertile = 8 windows x 2 heads on [128, 512] tiles (heads stacked on
partition halves -> 2-way PE quadrant packing via tile_position auto-derive).
"""

import numpy as np
from contextlib import ExitStack

import concourse.bass as bass
import concourse.tile as tile
from concourse import bacc
import concourse.mybir as mybir
from concourse.bass_utils import run_bass_kernel_spmd

WS, HEADS, HD, DISP = 8, 8, 64, 4
WS2 = WS * WS
B, C, H, W = 8, 512, 128, 128
NH = NW = 16
F16 = mybir.dt.float16
BF16 = mybir.dt.bfloat16
F32 = mybir.dt.float32


# ---------------------------------------------------------------- host consts
def _rel_bias(pe):
    idx = np.array([[i, j] for i in range(WS) for j in range(WS)])
    rel = idx[None, :, :] - idx[:, None, :] + (WS - 1)  # [ws2, ws2, 2]
    return pe[rel[:, :, 0], rel[:, :, 1]]  # [qi, ki]


def _mask_muls():
    # multiplicative {0,1} masks, [qi, ki]
    ul = np.ones((WS2, WS2), np.float32)
    ul[-DISP * WS:, :-DISP * WS] = 0.0
    ul[:-DISP * WS, -DISP * WS:] = 0.0
    lr = np.ones((WS, WS, WS, WS), np.float32)
    lr[:, -DISP:, :, :-DISP] = 0.0
    lr[:, :-DISP, :, -DISP:] = 0.0
    return ul, lr.reshape(WS2, WS2)


def _make_consts(pe, Wq, Wk, Wv, Wl):
    # u' = G x with G = Wq^T Wk / sqrt(hd):  s^T = (G x_k)^T x_q
    # matmul computes lhsT.T @ rhs, so lhsT = blockdiag(G)^T
    G = (Wq.T @ Wk / np.sqrt(HD)).astype(np.float32)      # [d, d']
    bdg = np.zeros((128, 128), np.float32)
    bdg[:64, :64] = G.T
    bdg[64:, 64:] = G.T
    Hm = (Wl @ Wv).astype(np.float32)                     # [f, d]
    ht2 = np.concatenate([Hm.T, Hm.T], axis=0)            # [128, 64]
    bias = _rel_bias(pe)                                  # [qi, ki]
    ul, lr = _mask_muls()
    ebT = np.exp(bias).T                                  # [ki, qi]
    ulT, lrT = ul.T, lr.T                                 # [ki, qi]
    # et[p, v, c]: p = 64*h + ki, c = 64*w + qi (w = window-in-supertile 0..7)
    et = np.empty((128, 4, 512), np.float32)
    for v in range(4):
        base = ebT * ulT if v >= 2 else ebT
        for w in range(8):
            blk = base * lrT if (v % 2 == 1 and w == 7) else base
            et[:64, v, 64 * w:64 * w + 64] = blk
            et[64:, v, 64 * w:64 * w + 64] = blk
    ones = np.ones((128, 64), np.float32)
    return bdg, ht2, et, ones


# ---------------------------------------------------------------- bass program
_CACHED_NC = None


def _build_nc():
    nc = bacc.Bacc("TRN2", target_bir_lowering=False, debug=False,
                   enable_asserts=False)
    xin = nc.dram_tensor("x16", [C, H, W], F16, kind="ExternalInput")
    sin = nc.dram_tensor("skip16", [C, H, W], F16, kind="ExternalInput")
    g2d = nc.dram_tensor("g2", [128, 128], F16, kind="ExternalInput")
    ht2d = nc.dram_tensor("ht2", [128, 64], F16, kind="ExternalInput")
    etd = nc.dram_tensor("et", [128, 4, 512], BF16, kind="ExternalInput")
    onesd = nc.dram_tensor("ones1", [128, 64], BF16, kind="ExternalInput")
    outd = nc.dram_tensor("out", [C, H, W], F32, kind="ExternalOutput")

    x_ap, s_ap, o_ap = xin.ap(), sin.ap(), outd.ap()

    with ExitStack() as ctx:
        tc = ctx.enter_context(tile.TileContext(nc))
        const = ctx.enter_context(tc.tile_pool(name="const", bufs=1))
        g2_sb = const.tile([128, 128], F16)
        nc.sync.dma_start(out=g2_sb, in_=g2d.ap())
        ht2_sb = const.tile([128, 64], F16)
        nc.sync.dma_start(out=ht2_sb, in_=ht2d.ap())
        et_sb = const.tile([128, 4, 512], BF16)
        nc.sync.dma_start(out=et_sb, in_=etd.ap())
        ones_sb = const.tile([128, 64], BF16)
        nc.sync.dma_start(out=ones_sb, in_=onesd.ap())

        rows = ctx.enter_context(tc.tile_pool(name="rows", bufs=3))
        work = ctx.enter_context(tc.tile_pool(name="work", bufs=3))
        pU = ctx.enter_context(tc.tile_pool(name="pU", bufs=2, space="PSUM"))
        pS = ctx.enter_context(tc.tile_pool(name="pS", bufs=2, space="PSUM"))
        pZ = ctx.enter_context(tc.tile_pool(name="pZ", bufs=1, space="PSUM"))
        pW = ctx.enter_context(tc.tile_pool(name="pW", bufs=2, space="PSUM"))
        pY = ctx.enter_context(tc.tile_pool(name="pY", bufs=1, space="PSUM"))

        for hp in range(4):
            cs = 128 * hp
            for wi in range(NH):
                x_row = rows.tile([128, 8, 128], F16, tag="xr")
                s_row = rows.tile([128, 8, 128], F16, tag="sr")
                for dst, src in ((x_row, x_ap), (s_row, s_ap)):
                    sc = src[cs:cs + 128]
                    if wi < 15:
                        r0 = 8 * wi + 4
                        nc.sync.dma_start(out=dst[:, :, 0:124],
                                          in_=sc[:, r0:r0 + 8, 4:128])
                        nc.sync.dma_start(out=dst[:, :, 124:128],
                                          in_=sc[:, r0:r0 + 8, 0:4])
                    else:
                        nc.sync.dma_start(out=dst[:, 0:4, 0:124],
                                          in_=sc[:, 124:128, 4:128])
                        nc.sync.dma_start(out=dst[:, 0:4, 124:128],
                                          in_=sc[:, 124:128, 0:4])
                        nc.sync.dma_start(out=dst[:, 4:8, 0:124],
                                          in_=sc[:, 0:4, 4:128])
                        nc.sync.dma_start(out=dst[:, 4:8, 124:128],
                                          in_=sc[:, 0:4, 0:4])
                for half in range(2):
                    _supertile(nc, tc, dict(
                        x_row=x_row, s_row=s_row, g2=g2_sb, ht2=ht2_sb,
                        et=et_sb, ones=ones_sb, work=work,
                        pU=pU, pS=pS, pZ=pZ, pW=pW, pY=pY,
                        o_ap=o_ap, hp=hp, wi=wi, half=half))
    return nc


def _supertile(nc, tc, a):
    hp, wi, half = a["hp"], a["wi"], a["half"]
    x_row, s_row = a["x_row"], a["s_row"]
    work = a["work"]
    c0 = 64 * half
    var = (2 if wi == 15 else 0) + half

    # u' = G x  -> psum cols (si, j); copy to SBUF window-grouped (w, si, sj)
    u_ps = a["pU"].tile([128, 512], F32, tag="u")
    nc.tensor.matmul(out=u_ps, lhsT=a["g2"], rhs=x_row[:, :, c0:c0 + 64],
                     start=True, stop=True)
    u_sb = work.tile([128, 512], F16, tag="u_sb")
    nc.scalar.copy(
        out=u_sb.rearrange("p (w si sj) -> p w si sj", w=8, si=8, sj=8),
        in_=u_ps.rearrange("p (si w sj) -> p w si sj", si=8, w=8, sj=8))

    # window-grouped skip copy (w1 stationary operand needs contiguous free)
    skw = work.tile([128, 512], F16, tag="skw")
    nc.vector.tensor_copy(
        skw.rearrange("p (w ti tj) -> p w ti tj", w=8, ti=8, tj=8),
        s_row[:, :, c0:c0 + 64].rearrange(
            "p ti (w tj) -> p w ti tj", w=8, tj=8))

    # s^T blocks: [64h+ki, 64w+qi]
    s_ps = a["pS"].tile([128, 512], F32, tag="s")
    for h in range(2):
        hb = 64 * h
        for w in range(8):
            nc.tensor.matmul(
                out=s_ps[hb:hb + 64, 64 * w:64 * w + 64],
                lhsT=u_sb[hb:hb + 64, 64 * w:64 * w + 64],
                rhs=x_row[hb:hb + 64, :, c0 + 8 * w:c0 + 8 * w + 8],
                start=True, stop=True)

    # p_hat = exp(s) * ET   (bf16)
    p_raw = work.tile([128, 512], BF16, tag="p_raw")
    nc.scalar.activation(out=p_raw, in_=s_ps,
                         func=mybir.ActivationFunctionType.Exp)
    p_m = work.tile([128, 512], BF16, tag="p_m")
    nc.vector.tensor_mul(p_m, p_raw, a["et"][:, var, :])

    # Z broadcast to all partitions: ones[64,64]^T @ p_hat
    zb_ps = a["pZ"].tile([128, 512], F32, tag="zb")
    for h in range(2):
        hb = 64 * h
        nc.tensor.matmul(out=zb_ps[hb:hb + 64, :],
                         lhsT=a["ones"][hb:hb + 64, :],
                         rhs=p_m[hb:hb + 64, :], start=True, stop=True)
    rz = work.tile([128, 512], F32, tag="rz")
    nc.vector.reciprocal(rz, zb_ps)

    # w1 = (H skip_w)^T blocks [64h+ki, 64w+f]
    w1_ps = a["pW"].tile([128, 512], F32, tag="w1")
    for h in range(2):
        hb = 64 * h
        for w in range(8):
            nc.tensor.matmul(
                out=w1_ps[hb:hb + 64, 64 * w:64 * w + 64],
                lhsT=skw[hb:hb + 64, 64 * w:64 * w + 64],
                rhs=a["ht2"][hb:hb + 64, :], start=True, stop=True)
    w1_sb = work.tile([128, 512], BF16, tag="w1_sb")
    nc.vector.tensor_copy(w1_sb, w1_ps)

    # y blocks [64h+f, 64w+qi]
    y_ps = a["pY"].tile([128, 512], F32, tag="y")
    for h in range(2):
        hb = 64 * h
        for w in range(8):
            nc.tensor.matmul(
                out=y_ps[hb:hb + 64, 64 * w:64 * w + 64],
                lhsT=w1_sb[hb:hb + 64, 64 * w:64 * w + 64],
                rhs=p_m[hb:hb + 64, 64 * w:64 * w + 64],
                start=True, stop=True)
    # normalize + permute to image-row-major (ti, w, tj) during PSUM->SBUF
    y_sb = work.tile([128, 8, 8, 8], F32, tag="y_sb")  # (ti, w, tj)
    nc.vector.tensor_mul(
        y_sb.rearrange("p ti w tj -> p w ti tj"),
        y_ps.rearrange("p (w ti tj) -> p w ti tj", w=8, ti=8, tj=8),
        rz.rearrange("p (w ti tj) -> p w ti tj", w=8, ti=8, tj=8))

    # scatter back to HBM with roll(+4,+4)
    o_ap = a["o_ap"]
    oc = o_ap[128 * hp:128 * hp + 128]

    def put(ti0, ti1, w0, w1_, tj0, tj1, rdst, cdst):
        ncols = (w1_ - w0 - 1) * 8 + (tj1 - tj0) if w1_ - w0 > 1 \
            else (tj1 - tj0)
        assert w1_ - w0 == 1 or (tj0, tj1) == (0, 8)
        nc.sync.dma_start(
            out=oc[:, rdst:rdst + (ti1 - ti0), cdst:cdst + ncols],
            in_=y_sb[:, ti0:ti1, w0:w1_, tj0:tj1])

    if wi < 15:
        rparts = [(0, 8, 8 * wi + 4)]
    else:
        rparts = [(0, 4, 124), (4, 8, 0)]
    for (ti0, ti1, rdst) in rparts:
        if half == 0:
            put(ti0, ti1, 0, 8, 0, 8, rdst, 4)
        else:
            put(ti0, ti1, 0, 7, 0, 8, rdst, 68)
            put(ti0, ti1, 7, 8, 0, 4, rdst, 124)
            put(ti0, ti1, 7, 8, 4, 8, rdst, 0)


def _get_nc():
    global _CACHED_NC
    if _CACHED_NC is None:
        _CACHED_NC = _build_nc()
        _CACHED_NC.compile()
    return _CACHED_NC


# ---------------------------------------------------------------- entry point
def _run(skip, x, pe, Wq, Wk, Wv, Wl, **spmd_kwargs):
    import ml_dtypes
    skip = np.asarray(skip, np.float32)
    x = np.asarray(x, np.float32)
    bdg, ht2, et, ones = _make_consts(np.asarray(pe, np.float32),
                                      np.asarray(Wq, np.float32),
                                      np.asarray(Wk, np.float32),
                                      np.asarray(Wv, np.float32),
                                      np.asarray(Wl, np.float32))
    x16 = x.astype(np.float16)
    skip16 = skip.astype(np.float16)
    bdg16 = bdg.astype(np.float16)
    ht216 = ht2.astype(np.float16)
    etb = et.astype(ml_dtypes.bfloat16)
    onesb = ones.astype(ml_dtypes.bfloat16)

    nc = _get_nc()
    in_maps = [dict(x16=x16[b], skip16=skip16[b], g2=bdg16, ht2=ht216,
                    et=etb, ones1=onesb) for b in range(B)]
    res = run_bass_kernel_spmd(nc, in_maps, core_ids=list(range(8)),
                               **spmd_kwargs)
    return np.stack([r["out"] for r in res.results], axis=0), res


def kernel(skip, x, pe, Wq, Wk, Wv, Wl):
    out, _ = _run(skip, x, pe, Wq, Wk, Wv, Wl)
    return out


# revision 12
# speedup vs baseline: 3.1701x; 3.1701x over previous
"""MultiHeadCrossWindowAttention Bass/Tile kernel for 8 trn2 NeuronCores.

Strategy (data-parallel over batch, 1 batch item per core):
  - Fold Wq/Wk into G = Wq.T @ Wk / sqrt(hd)  (shared across heads):
      scores^T = x_w^T G^T x_w  per window  ->  u = G^T x (streaming matmul),
      s^T[ki,qi] = sum_d x[d,ki] u[d,qi]  (per-window matmul, lhsT = raw x).
  - Fold Wl @ Wv into H: y = (H skip_w) p_hat:
      w1 = (H skip_w)^T  via lhsT = skip_w, rhs = H^T,
      y[f,qi] = sum_ki w1[ki,f] p_hat[ki,qi]  (channel-major output).
  - softmax: p = exp(s^T) * ET (ET = exp(bias+mask) tables, multiplicative),
      Z broadcast to all partitions via all-ones [64,64] stationary matmul,
      y normalized during the PSUM->SBUF copy (tensor_mul with 1/Z).
  - Shifted-window roll (+-4) folded into HBM<->SBUF DMA access patterns.

Layout per core: 4 head-pairs x 16 window-rows x 2 halves = 128 supertiles,
each supertile = 8 windows x 2 heads on [128, 512] tiles (heads stacked on
partition halves -> 2-way PE quadrant packing via tile_position auto-derive).
"""

import numpy as np
from contextlib import ExitStack

import concourse.bass as bass
import concourse.tile as tile
from concourse import bacc
import concourse.mybir as mybir
from concourse.bass_utils import run_bass_kernel_spmd

WS, HEADS, HD, DISP = 8, 8, 64, 4
WS2 = WS * WS
B, C, H, W = 8, 512, 128, 128
NH = NW = 16
F16 = mybir.dt.float16
BF16 = mybir.dt.bfloat16
F32 = mybir.dt.float32


# ---------------------------------------------------------------- host consts
def _rel_bias(pe):
    idx = np.array([[i, j] for i in range(WS) for j in range(WS)])
    rel = idx[None, :, :] - idx[:, None, :] + (WS - 1)  # [ws2, ws2, 2]
    return pe[rel[:, :, 0], rel[:, :, 1]]  # [qi, ki]


def _mask_muls():
    # multiplicative {0,1} masks, [qi, ki]
    ul = np.ones((WS2, WS2), np.float32)
    ul[-DISP * WS:, :-DISP * WS] = 0.0
    ul[:-DISP * WS, -DISP * WS:] = 0.0
    lr = np.ones((WS, WS, WS, WS), np.float32)
    lr[:, -DISP:, :, :-DISP] = 0.0
    lr[:, :-DISP, :, -DISP:] = 0.0
    return ul, lr.reshape(WS2, WS2)


def _make_consts(pe, Wq, Wk, Wv, Wl):
    # u' = G x with G = Wq^T Wk / sqrt(hd):  s^T = (G x_k)^T x_q
    # matmul computes lhsT.T @ rhs, so lhsT = blockdiag(G)^T
    G = (Wq.T @ Wk / np.sqrt(HD)).astype(np.float32)      # [d, d']
    bdg = np.zeros((128, 128), np.float32)
    bdg[:64, :64] = G.T
    bdg[64:, 64:] = G.T
    Hm = (Wl @ Wv).astype(np.float32)                     # [f, d]
    ht2 = np.concatenate([Hm.T, Hm.T], axis=0)            # [128, 64]
    bias = _rel_bias(pe)                                  # [qi, ki]
    ul, lr = _mask_muls()
    ebT = np.exp(bias).T                                  # [ki, qi]
    ulT, lrT = ul.T, lr.T                                 # [ki, qi]
    # et[p, v, c]: p = 64*h + ki, c = 64*w + qi (w = window-in-supertile 0..7)
    et = np.empty((128, 4, 512), np.float32)
    for v in range(4):
        base = ebT * ulT if v >= 2 else ebT
        for w in range(8):
            blk = base * lrT if (v % 2 == 1 and w == 7) else base
            et[:64, v, 64 * w:64 * w + 64] = blk
            et[64:, v, 64 * w:64 * w + 64] = blk
    ones = np.ones((128, 64), np.float32)
    return bdg, ht2, et, ones


# ---------------------------------------------------------------- bass program
_CACHED_NC = None


def _build_nc(repeat=1):
    nc = bacc.Bacc("TRN2", target_bir_lowering=False, debug=False,
                   enable_asserts=False)
    xin = nc.dram_tensor("x16", [C, H, W], F16, kind="ExternalInput")
    sin = nc.dram_tensor("skip16", [C, H, W], F16, kind="ExternalInput")
    g2d = nc.dram_tensor("g2", [128, 128], F16, kind="ExternalInput")
    ht2d = nc.dram_tensor("ht2", [128, 64], F16, kind="ExternalInput")
    etd = nc.dram_tensor("et", [128, 4, 512], BF16, kind="ExternalInput")
    onesd = nc.dram_tensor("ones1", [128, 64], BF16, kind="ExternalInput")
    outd = nc.dram_tensor("out", [C, H, W], F32, kind="ExternalOutput")

    x_ap, s_ap, o_ap = xin.ap(), sin.ap(), outd.ap()

    with ExitStack() as ctx:
        tc = ctx.enter_context(tile.TileContext(nc))
        const = ctx.enter_context(tc.tile_pool(name="const", bufs=1))
        g2_sb = const.tile([128, 128], F16)
        nc.sync.dma_start(out=g2_sb, in_=g2d.ap())
        ht2_sb = const.tile([128, 64], F16)
        nc.sync.dma_start(out=ht2_sb, in_=ht2d.ap())
        et_sb = const.tile([128, 4, 512], BF16)
        nc.sync.dma_start(out=et_sb, in_=etd.ap())
        ones_sb = const.tile([128, 64], BF16)
        nc.sync.dma_start(out=ones_sb, in_=onesd.ap())

        rows = ctx.enter_context(tc.tile_pool(name="rows", bufs=3))
        work = ctx.enter_context(tc.tile_pool(name="work", bufs=3))
        pU = ctx.enter_context(tc.tile_pool(name="pU", bufs=2, space="PSUM"))
        pS = ctx.enter_context(tc.tile_pool(name="pS", bufs=2, space="PSUM"))
        pZ = ctx.enter_context(tc.tile_pool(name="pZ", bufs=1, space="PSUM"))
        pW = ctx.enter_context(tc.tile_pool(name="pW", bufs=2, space="PSUM"))
        pY = ctx.enter_context(tc.tile_pool(name="pY", bufs=1, space="PSUM"))

        for rep in range(repeat):
          for hp in range(4):
            cs = 128 * hp
            for wi in range(NH):
                x_row = rows.tile([128, 8, 128], F16, tag="xr")
                s_row = rows.tile([128, 8, 128], F16, tag="sr")
                for dst, src in ((x_row, x_ap), (s_row, s_ap)):
                    sc = src[cs:cs + 128]
                    if wi < 15:
                        r0 = 8 * wi + 4
                        nc.sync.dma_start(out=dst[:, :, 0:124],
                                          in_=sc[:, r0:r0 + 8, 4:128])
                        nc.sync.dma_start(out=dst[:, :, 124:128],
                                          in_=sc[:, r0:r0 + 8, 0:4])
                    else:
                        nc.sync.dma_start(out=dst[:, 0:4, 0:124],
                                          in_=sc[:, 124:128, 4:128])
                        nc.sync.dma_start(out=dst[:, 0:4, 124:128],
                                          in_=sc[:, 124:128, 0:4])
                        nc.sync.dma_start(out=dst[:, 4:8, 0:124],
                                          in_=sc[:, 0:4, 4:128])
                        nc.sync.dma_start(out=dst[:, 4:8, 124:128],
                                          in_=sc[:, 0:4, 0:4])
                for half in range(2):
                    _supertile(nc, tc, dict(
                        x_row=x_row, s_row=s_row, g2=g2_sb, ht2=ht2_sb,
                        et=et_sb, ones=ones_sb, work=work,
                        pU=pU, pS=pS, pZ=pZ, pW=pW, pY=pY,
                        o_ap=o_ap, hp=hp, wi=wi, half=half))
    return nc


def _supertile(nc, tc, a):
    hp, wi, half = a["hp"], a["wi"], a["half"]
    x_row, s_row = a["x_row"], a["s_row"]
    work = a["work"]
    c0 = 64 * half
    var = (2 if wi == 15 else 0) + half

    # u' = G x  -> psum cols (si, j); copy to SBUF window-grouped (w, si, sj)
    u_ps = a["pU"].tile([128, 512], F32, tag="u")
    nc.tensor.matmul(out=u_ps, lhsT=a["g2"], rhs=x_row[:, :, c0:c0 + 64],
                     start=True, stop=True)
    u_sb = work.tile([128, 512], F16, tag="u_sb")
    nc.scalar.copy(
        out=u_sb.rearrange("p (w si sj) -> p w si sj", w=8, si=8, sj=8),
        in_=u_ps.rearrange("p (si w sj) -> p w si sj", si=8, w=8, sj=8))

    # window-grouped skip copy (w1 stationary operand needs contiguous free)
    skw = work.tile([128, 512], F16, tag="skw")
    nc.vector.tensor_copy(
        skw.rearrange("p (w ti tj) -> p w ti tj", w=8, ti=8, tj=8),
        s_row[:, :, c0:c0 + 64].rearrange(
            "p ti (w tj) -> p w ti tj", w=8, tj=8))

    # s^T blocks: [64h+ki, 64w+qi]
    s_ps = a["pS"].tile([128, 512], F32, tag="s")
    for h in range(2):
        hb = 64 * h
        for w in range(8):
            nc.tensor.matmul(
                out=s_ps[hb:hb + 64, 64 * w:64 * w + 64],
                lhsT=u_sb[hb:hb + 64, 64 * w:64 * w + 64],
                rhs=x_row[hb:hb + 64, :, c0 + 8 * w:c0 + 8 * w + 8],
                start=True, stop=True)

    # p_hat = exp(s) * ET   (bf16)
    p_raw = work.tile([128, 512], BF16, tag="p_raw")
    nc.scalar.activation(out=p_raw, in_=s_ps,
                         func=mybir.ActivationFunctionType.Exp)
    p_m = work.tile([128, 512], BF16, tag="p_m")
    nc.vector.tensor_mul(p_m, p_raw, a["et"][:, var, :])

    # Z broadcast to all partitions: ones[64,64]^T @ p_hat
    zb_ps = a["pZ"].tile([128, 512], F32, tag="zb")
    for h in range(2):
        hb = 64 * h
        nc.tensor.matmul(out=zb_ps[hb:hb + 64, :],
                         lhsT=a["ones"][hb:hb + 64, :],
                         rhs=p_m[hb:hb + 64, :], start=True, stop=True)
    rz = work.tile([128, 512], F32, tag="rz")
    nc.vector.reciprocal(rz, zb_ps)

    # w1 = (H skip_w)^T blocks [64h+ki, 64w+f]
    w1_ps = a["pW"].tile([128, 512], F32, tag="w1")
    for h in range(2):
        hb = 64 * h
        for w in range(8):
            nc.tensor.matmul(
                out=w1_ps[hb:hb + 64, 64 * w:64 * w + 64],
                lhsT=skw[hb:hb + 64, 64 * w:64 * w + 64],
                rhs=a["ht2"][hb:hb + 64, :], start=True, stop=True)
    w1_sb = work.tile([128, 512], BF16, tag="w1_sb")
    nc.vector.tensor_copy(w1_sb, w1_ps)

    # y blocks [64h+f, 64w+qi]
    y_ps = a["pY"].tile([128, 512], F32, tag="y")
    for h in range(2):
        hb = 64 * h
        for w in range(8):
            nc.tensor.matmul(
                out=y_ps[hb:hb + 64, 64 * w:64 * w + 64],
                lhsT=w1_sb[hb:hb + 64, 64 * w:64 * w + 64],
                rhs=p_m[hb:hb + 64, 64 * w:64 * w + 64],
                start=True, stop=True)
    # normalize + permute to image-row-major (ti, w, tj) during PSUM->SBUF
    y_sb = work.tile([128, 8, 8, 8], F32, tag="y_sb")  # (ti, w, tj)
    nc.vector.tensor_mul(
        y_sb.rearrange("p ti w tj -> p w ti tj"),
        y_ps.rearrange("p (w ti tj) -> p w ti tj", w=8, ti=8, tj=8),
        rz.rearrange("p (w ti tj) -> p w ti tj", w=8, ti=8, tj=8))

    # scatter back to HBM with roll(+4,+4)
    o_ap = a["o_ap"]
    oc = o_ap[128 * hp:128 * hp + 128]

    def put(ti0, ti1, w0, w1_, tj0, tj1, rdst, cdst):
        ncols = (w1_ - w0 - 1) * 8 + (tj1 - tj0) if w1_ - w0 > 1 \
            else (tj1 - tj0)
        assert w1_ - w0 == 1 or (tj0, tj1) == (0, 8)
        nc.sync.dma_start(
            out=oc[:, rdst:rdst + (ti1 - ti0), cdst:cdst + ncols],
            in_=y_sb[:, ti0:ti1, w0:w1_, tj0:tj1])

    if wi < 15:
        rparts = [(0, 8, 8 * wi + 4)]
    else:
        rparts = [(0, 4, 124), (4, 8, 0)]
    for (ti0, ti1, rdst) in rparts:
        if half == 0:
            put(ti0, ti1, 0, 8, 0, 8, rdst, 4)
        else:
            put(ti0, ti1, 0, 7, 0, 8, rdst, 68)
            put(ti0, ti1, 7, 8, 0, 4, rdst, 124)
            put(ti0, ti1, 7, 8, 4, 8, rdst, 0)


def _get_nc():
    global _CACHED_NC
    if _CACHED_NC is None:
        _CACHED_NC = _build_nc()
        _CACHED_NC.compile()
    return _CACHED_NC


# ---------------------------------------------------------------- entry point
def _run(skip, x, pe, Wq, Wk, Wv, Wl, **spmd_kwargs):
    import ml_dtypes
    skip = np.asarray(skip, np.float32)
    x = np.asarray(x, np.float32)
    bdg, ht2, et, ones = _make_consts(np.asarray(pe, np.float32),
                                      np.asarray(Wq, np.float32),
                                      np.asarray(Wk, np.float32),
                                      np.asarray(Wv, np.float32),
                                      np.asarray(Wl, np.float32))
    x16 = x.astype(np.float16)
    skip16 = skip.astype(np.float16)
    bdg16 = bdg.astype(np.float16)
    ht216 = ht2.astype(np.float16)
    etb = et.astype(ml_dtypes.bfloat16)
    onesb = ones.astype(ml_dtypes.bfloat16)

    nc = _get_nc()
    in_maps = [dict(x16=x16[b], skip16=skip16[b], g2=bdg16, ht2=ht216,
                    et=etb, ones1=onesb) for b in range(B)]
    res = run_bass_kernel_spmd(nc, in_maps, core_ids=list(range(8)),
                               **spmd_kwargs)
    return np.stack([r["out"] for r in res.results], axis=0), res


def kernel(skip, x, pe, Wq, Wk, Wv, Wl):
    out, _ = _run(skip, x, pe, Wq, Wk, Wv, Wl)
    return out


# revision 13
# speedup vs baseline: 4.7084x; 1.4852x over previous
"""MultiHeadCrossWindowAttention Bass/Tile kernel for 8 trn2 NeuronCores.

Strategy (data-parallel over batch, 1 batch item per core):
  - Fold Wq/Wk into G = Wq.T @ Wk / sqrt(hd)  (shared across heads):
      scores^T = x_w^T G^T x_w  per window  ->  u = G^T x (streaming matmul),
      s^T[ki,qi] = sum_d x[d,ki] u[d,qi]  (per-window matmul, lhsT = raw x).
  - Fold Wl @ Wv into H: y = (H skip_w) p_hat:
      w1 = (H skip_w)^T  via lhsT = skip_w, rhs = H^T,
      y[f,qi] = sum_ki w1[ki,f] p_hat[ki,qi]  (channel-major output).
  - softmax: p = exp(s^T) * ET (ET = exp(bias+mask) tables, multiplicative),
      Z broadcast to all partitions via all-ones [64,64] stationary matmul,
      y normalized during the PSUM->SBUF copy (tensor_mul with 1/Z).
  - Shifted-window roll (+-4) folded into HBM<->SBUF DMA access patterns.

Layout per core: 4 head-pairs x 16 window-rows x 2 halves = 128 supertiles,
each supertile = 8 windows x 2 heads on [128, 512] tiles (heads stacked on
partition halves -> 2-way PE quadrant packing via tile_position auto-derive).
"""

import numpy as np
from contextlib import ExitStack

import concourse.bass as bass
import concourse.tile as tile
from concourse import bacc
import concourse.mybir as mybir
from concourse.bass_utils import run_bass_kernel_spmd

WS, HEADS, HD, DISP = 8, 8, 64, 4
WS2 = WS * WS
B, C, H, W = 8, 512, 128, 128
NH = NW = 16
F16 = mybir.dt.float16
BF16 = mybir.dt.bfloat16
F32 = mybir.dt.float32


# ---------------------------------------------------------------- host consts
def _rel_bias(pe):
    idx = np.array([[i, j] for i in range(WS) for j in range(WS)])
    rel = idx[None, :, :] - idx[:, None, :] + (WS - 1)  # [ws2, ws2, 2]
    return pe[rel[:, :, 0], rel[:, :, 1]]  # [qi, ki]


def _mask_muls():
    # multiplicative {0,1} masks, [qi, ki]
    ul = np.ones((WS2, WS2), np.float32)
    ul[-DISP * WS:, :-DISP * WS] = 0.0
    ul[:-DISP * WS, -DISP * WS:] = 0.0
    lr = np.ones((WS, WS, WS, WS), np.float32)
    lr[:, -DISP:, :, :-DISP] = 0.0
    lr[:, :-DISP, :, -DISP:] = 0.0
    return ul, lr.reshape(WS2, WS2)


def _make_consts(pe, Wq, Wk, Wv, Wl):
    # u' = G x with G = Wq^T Wk / sqrt(hd):  s^T = (G x_k)^T x_q
    # matmul computes lhsT.T @ rhs, so lhsT = blockdiag(G)^T
    G = (Wq.T @ Wk / np.sqrt(HD)).astype(np.float32)      # [d, d']
    bdg = np.zeros((128, 128), np.float32)
    bdg[:64, :64] = G.T
    bdg[64:, 64:] = G.T
    Hm = (Wl @ Wv).astype(np.float32)                     # [f, d]
    ht2 = np.concatenate([Hm.T, Hm.T], axis=0)            # [128, 64]
    bias = _rel_bias(pe)                                  # [qi, ki]
    ul, lr = _mask_muls()
    ebT = np.exp(bias).T                                  # [ki, qi]
    ulT, lrT = ul.T, lr.T                                 # [ki, qi]
    # et[p, v, c]: p = 64*h + ki, c = 64*w + qi (w = window-in-supertile 0..7)
    et = np.empty((128, 4, 512), np.float32)
    for v in range(4):
        base = ebT * ulT if v >= 2 else ebT
        for w in range(8):
            blk = base * lrT if (v % 2 == 1 and w == 7) else base
            et[:64, v, 64 * w:64 * w + 64] = blk
            et[64:, v, 64 * w:64 * w + 64] = blk
    ones = np.ones((128, 64), np.float32)
    return bdg, ht2, et, ones


# ---------------------------------------------------------------- bass program
_CACHED_NC = None


def _build_nc(repeat=1):
    nc = bacc.Bacc("TRN2", target_bir_lowering=False, debug=False,
                   enable_asserts=False)
    xin = nc.dram_tensor("x16", [C, H, W], F16, kind="ExternalInput")
    sin = nc.dram_tensor("skip16", [C, H, W], F16, kind="ExternalInput")
    g2d = nc.dram_tensor("g2", [128, 128], F16, kind="ExternalInput")
    ht2d = nc.dram_tensor("ht2", [128, 64], F16, kind="ExternalInput")
    etd = nc.dram_tensor("et", [128, 4, 512], BF16, kind="ExternalInput")
    onesd = nc.dram_tensor("ones1", [128, 64], BF16, kind="ExternalInput")
    outd = nc.dram_tensor("out", [C, H, W], F32, kind="ExternalOutput")

    x_ap, s_ap, o_ap = xin.ap(), sin.ap(), outd.ap()

    with ExitStack() as ctx:
        tc = ctx.enter_context(tile.TileContext(nc))
        const = ctx.enter_context(tc.tile_pool(name="const", bufs=1))
        g2_sb = const.tile([128, 128], F16)
        nc.sync.dma_start(out=g2_sb, in_=g2d.ap())
        ht2_sb = const.tile([128, 64], F16)
        nc.sync.dma_start(out=ht2_sb, in_=ht2d.ap())
        et_sb = const.tile([128, 4, 512], BF16)
        nc.sync.dma_start(out=et_sb, in_=etd.ap())
        ones_sb = const.tile([128, 64], BF16)
        nc.sync.dma_start(out=ones_sb, in_=onesd.ap())

        rows = ctx.enter_context(tc.tile_pool(name="rows", bufs=3))
        work = ctx.enter_context(tc.tile_pool(name="work", bufs=3))
        pU = ctx.enter_context(tc.tile_pool(name="pU", bufs=1, space="PSUM"))
        pS = ctx.enter_context(tc.tile_pool(name="pS", bufs=1, space="PSUM"))
        pZ = ctx.enter_context(tc.tile_pool(name="pZ", bufs=2, space="PSUM"))
        pW = ctx.enter_context(tc.tile_pool(name="pW", bufs=2, space="PSUM"))
        pY = ctx.enter_context(tc.tile_pool(name="pY", bufs=2, space="PSUM"))

        for rep in range(repeat):
          for hp in range(4):
            cs = 128 * hp
            for wi in range(NH):
                x_row = rows.tile([128, 8, 128], F16, tag="xr")
                s_row = rows.tile([128, 8, 128], F16, tag="sr")
                for dst, src in ((x_row, x_ap), (s_row, s_ap)):
                    sc = src[cs:cs + 128]
                    if wi < 15:
                        r0 = 8 * wi + 4
                        nc.sync.dma_start(out=dst[:, :, 0:124],
                                          in_=sc[:, r0:r0 + 8, 4:128])
                        nc.sync.dma_start(out=dst[:, :, 124:128],
                                          in_=sc[:, r0:r0 + 8, 0:4])
                    else:
                        nc.sync.dma_start(out=dst[:, 0:4, 0:124],
                                          in_=sc[:, 124:128, 4:128])
                        nc.sync.dma_start(out=dst[:, 0:4, 124:128],
                                          in_=sc[:, 124:128, 0:4])
                        nc.sync.dma_start(out=dst[:, 4:8, 0:124],
                                          in_=sc[:, 0:4, 4:128])
                        nc.sync.dma_start(out=dst[:, 4:8, 124:128],
                                          in_=sc[:, 0:4, 0:4])
                for half in range(2):
                    _supertile(nc, tc, dict(
                        x_row=x_row, s_row=s_row, g2=g2_sb, ht2=ht2_sb,
                        et=et_sb, ones=ones_sb, work=work,
                        pU=pU, pS=pS, pZ=pZ, pW=pW, pY=pY,
                        o_ap=o_ap, hp=hp, wi=wi, half=half))
    return nc


def _supertile(nc, tc, a):
    hp, wi, half = a["hp"], a["wi"], a["half"]
    x_row, s_row = a["x_row"], a["s_row"]
    work = a["work"]
    c0 = 64 * half
    var = (2 if wi == 15 else 0) + half

    # u' = G x  -> psum cols (si, j); copy to SBUF window-grouped (w, si, sj)
    u_ps = a["pU"].tile([128, 512], F32, tag="u")
    nc.tensor.matmul(out=u_ps, lhsT=a["g2"], rhs=x_row[:, :, c0:c0 + 64],
                     start=True, stop=True)
    u_sb = work.tile([128, 512], F16, tag="u_sb")
    nc.scalar.copy(
        out=u_sb.rearrange("p (w si sj) -> p w si sj", w=8, si=8, sj=8),
        in_=u_ps.rearrange("p (si w sj) -> p w si sj", si=8, w=8, sj=8))

    # window-grouped skip copy (w1 stationary operand needs contiguous free)
    skw = work.tile([128, 512], F16, tag="skw")
    nc.vector.tensor_copy(
        skw.rearrange("p (w ti tj) -> p w ti tj", w=8, ti=8, tj=8),
        s_row[:, :, c0:c0 + 64].rearrange(
            "p ti (w tj) -> p w ti tj", w=8, tj=8))

    # s^T blocks: [64h+ki, 64w+qi]
    s_ps = a["pS"].tile([128, 512], F32, tag="s")
    for h in range(2):
        hb = 64 * h
        for w in range(8):
            nc.tensor.matmul(
                out=s_ps[hb:hb + 64, 64 * w:64 * w + 64],
                lhsT=u_sb[hb:hb + 64, 64 * w:64 * w + 64],
                rhs=x_row[hb:hb + 64, :, c0 + 8 * w:c0 + 8 * w + 8],
                start=True, stop=True)

    # p_hat = exp(s) * ET   (bf16)
    p_raw = work.tile([128, 512], BF16, tag="p_raw")
    nc.scalar.activation(out=p_raw, in_=s_ps,
                         func=mybir.ActivationFunctionType.Exp)
    p_m = work.tile([128, 512], BF16, tag="p_m")
    nc.vector.tensor_mul(p_m, p_raw, a["et"][:, var, :])

    # Z broadcast to all partitions: ones[64,64]^T @ p_hat
    zb_ps = a["pZ"].tile([128, 512], F32, tag="zb")
    for h in range(2):
        hb = 64 * h
        nc.tensor.matmul(out=zb_ps[hb:hb + 64, :],
                         lhsT=a["ones"][hb:hb + 64, :],
                         rhs=p_m[hb:hb + 64, :], start=True, stop=True)
    rz = work.tile([128, 512], F32, tag="rz")
    nc.vector.reciprocal(rz, zb_ps)

    # w1 = (H skip_w)^T blocks [64h+ki, 64w+f]
    w1_ps = a["pW"].tile([128, 512], F32, tag="w1")
    for h in range(2):
        hb = 64 * h
        for w in range(8):
            nc.tensor.matmul(
                out=w1_ps[hb:hb + 64, 64 * w:64 * w + 64],
                lhsT=skw[hb:hb + 64, 64 * w:64 * w + 64],
                rhs=a["ht2"][hb:hb + 64, :], start=True, stop=True)
    w1_sb = work.tile([128, 512], BF16, tag="w1_sb")
    nc.vector.tensor_copy(w1_sb, w1_ps)

    # y blocks [64h+f, 64w+qi]
    y_ps = a["pY"].tile([128, 512], F32, tag="y")
    for h in range(2):
        hb = 64 * h
        for w in range(8):
            nc.tensor.matmul(
                out=y_ps[hb:hb + 64, 64 * w:64 * w + 64],
                lhsT=w1_sb[hb:hb + 64, 64 * w:64 * w + 64],
                rhs=p_m[hb:hb + 64, 64 * w:64 * w + 64],
                start=True, stop=True)
    # normalize + permute to image-row-major (ti, w, tj) during PSUM->SBUF
    y_sb = work.tile([128, 8, 8, 8], F32, tag="y_sb")  # (ti, w, tj)
    nc.vector.tensor_mul(
        y_sb.rearrange("p ti w tj -> p w ti tj"),
        y_ps.rearrange("p (w ti tj) -> p w ti tj", w=8, ti=8, tj=8),
        rz.rearrange("p (w ti tj) -> p w ti tj", w=8, ti=8, tj=8))

    # scatter back to HBM with roll(+4,+4)
    o_ap = a["o_ap"]
    oc = o_ap[128 * hp:128 * hp + 128]

    def put(ti0, ti1, w0, w1_, tj0, tj1, rdst, cdst):
        ncols = (w1_ - w0 - 1) * 8 + (tj1 - tj0) if w1_ - w0 > 1 \
            else (tj1 - tj0)
        assert w1_ - w0 == 1 or (tj0, tj1) == (0, 8)
        nc.sync.dma_start(
            out=oc[:, rdst:rdst + (ti1 - ti0), cdst:cdst + ncols],
            in_=y_sb[:, ti0:ti1, w0:w1_, tj0:tj1])

    if wi < 15:
        rparts = [(0, 8, 8 * wi + 4)]
    else:
        rparts = [(0, 4, 124), (4, 8, 0)]
    for (ti0, ti1, rdst) in rparts:
        if half == 0:
            put(ti0, ti1, 0, 8, 0, 8, rdst, 4)
        else:
            put(ti0, ti1, 0, 7, 0, 8, rdst, 68)
            put(ti0, ti1, 7, 8, 0, 4, rdst, 124)
            put(ti0, ti1, 7, 8, 4, 8, rdst, 0)


def _get_nc():
    global _CACHED_NC
    if _CACHED_NC is None:
        _CACHED_NC = _build_nc()
        _CACHED_NC.compile()
    return _CACHED_NC


# ---------------------------------------------------------------- entry point
def _run(skip, x, pe, Wq, Wk, Wv, Wl, **spmd_kwargs):
    import ml_dtypes
    skip = np.asarray(skip, np.float32)
    x = np.asarray(x, np.float32)
    bdg, ht2, et, ones = _make_consts(np.asarray(pe, np.float32),
                                      np.asarray(Wq, np.float32),
                                      np.asarray(Wk, np.float32),
                                      np.asarray(Wv, np.float32),
                                      np.asarray(Wl, np.float32))
    x16 = x.astype(np.float16)
    skip16 = skip.astype(np.float16)
    bdg16 = bdg.astype(np.float16)
    ht216 = ht2.astype(np.float16)
    etb = et.astype(ml_dtypes.bfloat16)
    onesb = ones.astype(ml_dtypes.bfloat16)

    nc = _get_nc()
    in_maps = [dict(x16=x16[b], skip16=skip16[b], g2=bdg16, ht2=ht216,
                    et=etb, ones1=onesb) for b in range(B)]
    res = run_bass_kernel_spmd(nc, in_maps, core_ids=list(range(8)),
                               **spmd_kwargs)
    return np.stack([r["out"] for r in res.results], axis=0), res


def kernel(skip, x, pe, Wq, Wk, Wv, Wl):
    out, _ = _run(skip, x, pe, Wq, Wk, Wv, Wl)
    return out


# revision 15
# speedup vs baseline: 14.1834x; 3.0124x over previous
"""MultiHeadCrossWindowAttention Bass/Tile kernel for 8 trn2 NeuronCores.

Strategy (data-parallel over batch, 1 batch item per core):
  - Fold Wq/Wk into G = Wq.T @ Wk / sqrt(hd)  (shared across heads):
      scores^T = x_w^T G^T x_w  per window  ->  u = G^T x (streaming matmul),
      s^T[ki,qi] = sum_d x[d,ki] u[d,qi]  (per-window matmul, lhsT = raw x).
  - Fold Wl @ Wv into H: y = (H skip_w) p_hat:
      w1 = (H skip_w)^T  via lhsT = skip_w, rhs = H^T,
      y[f,qi] = sum_ki w1[ki,f] p_hat[ki,qi]  (channel-major output).
  - softmax: p = exp(s^T) * ET (ET = exp(bias+mask) tables, multiplicative),
      Z broadcast to all partitions via all-ones [64,64] stationary matmul,
      y normalized during the PSUM->SBUF copy (tensor_mul with 1/Z).
  - Shifted-window roll (+-4) folded into HBM<->SBUF DMA access patterns.

Layout per core: 4 head-pairs x 16 window-rows x 2 halves = 128 supertiles,
each supertile = 8 windows x 2 heads on [128, 512] tiles (heads stacked on
partition halves -> 2-way PE quadrant packing via tile_position auto-derive).
"""

import numpy as np
from contextlib import ExitStack

import concourse.bass as bass
import concourse.tile as tile
from concourse import bacc
import concourse.mybir as mybir
from concourse.bass_utils import run_bass_kernel_spmd

WS, HEADS, HD, DISP = 8, 8, 64, 4
WS2 = WS * WS
B, C, H, W = 8, 512, 128, 128
NH = NW = 16
F16 = mybir.dt.float16
BF16 = mybir.dt.bfloat16
F32 = mybir.dt.float32


# ---------------------------------------------------------------- host consts
def _rel_bias(pe):
    idx = np.array([[i, j] for i in range(WS) for j in range(WS)])
    rel = idx[None, :, :] - idx[:, None, :] + (WS - 1)  # [ws2, ws2, 2]
    return pe[rel[:, :, 0], rel[:, :, 1]]  # [qi, ki]


def _mask_muls():
    # multiplicative {0,1} masks, [qi, ki]
    ul = np.ones((WS2, WS2), np.float32)
    ul[-DISP * WS:, :-DISP * WS] = 0.0
    ul[:-DISP * WS, -DISP * WS:] = 0.0
    lr = np.ones((WS, WS, WS, WS), np.float32)
    lr[:, -DISP:, :, :-DISP] = 0.0
    lr[:, :-DISP, :, -DISP:] = 0.0
    return ul, lr.reshape(WS2, WS2)


def _make_consts(pe, Wq, Wk, Wv, Wl):
    # u' = G x with G = Wq^T Wk / sqrt(hd):  s^T = (G x_k)^T x_q
    # matmul computes lhsT.T @ rhs, so lhsT = blockdiag(G)^T
    G = (Wq.T @ Wk / np.sqrt(HD)).astype(np.float32)      # [d, d']
    bdg = np.zeros((128, 128), np.float32)
    bdg[:64, :64] = G.T
    bdg[64:, 64:] = G.T
    Hm = (Wl @ Wv).astype(np.float32)                     # [f, d]
    ht2 = np.concatenate([Hm.T, Hm.T], axis=0)            # [128, 64]
    bias = _rel_bias(pe)                                  # [qi, ki]
    ul, lr = _mask_muls()
    ebT = np.exp(bias).T                                  # [ki, qi]
    ulT, lrT = ul.T, lr.T                                 # [ki, qi]
    # et[p, v, c]: p = 64*h + ki, c = 64*w + qi (w = window-in-supertile 0..7)
    et = np.empty((128, 4, 512), np.float32)
    for v in range(4):
        base = ebT * ulT if v >= 2 else ebT
        for w in range(8):
            blk = base * lrT if (v % 2 == 1 and w == 7) else base
            et[:64, v, 64 * w:64 * w + 64] = blk
            et[64:, v, 64 * w:64 * w + 64] = blk
    ones = np.ones((128, 64), np.float32)
    return bdg, ht2, et, ones


# ---------------------------------------------------------------- bass program
_CACHED_NC = None


def _build_nc(repeat=1):
    nc = bacc.Bacc("TRN2", target_bir_lowering=False, debug=False,
                   enable_asserts=False)
    xin = nc.dram_tensor("x16", [C, H, W], F16, kind="ExternalInput")
    sin = nc.dram_tensor("skip16", [C, H, W], F16, kind="ExternalInput")
    g2d = nc.dram_tensor("g2", [128, 128], F16, kind="ExternalInput")
    ht2d = nc.dram_tensor("ht2", [128, 64], F16, kind="ExternalInput")
    etd = nc.dram_tensor("et", [128, 4, 512], BF16, kind="ExternalInput")
    onesd = nc.dram_tensor("ones1", [128, 64], BF16, kind="ExternalInput")
    outd = nc.dram_tensor("out", [C, H, W], F32, kind="ExternalOutput")

    x_ap, s_ap, o_ap = xin.ap(), sin.ap(), outd.ap()

    with ExitStack() as ctx:
        tc = ctx.enter_context(tile.TileContext(nc))
        const = ctx.enter_context(tc.tile_pool(name="const", bufs=1))
        g2_sb = const.tile([128, 128], F16)
        nc.sync.dma_start(out=g2_sb, in_=g2d.ap())
        ht2_sb = const.tile([128, 64], F16)
        nc.sync.dma_start(out=ht2_sb, in_=ht2d.ap())
        et_sb = const.tile([128, 4, 512], BF16)
        nc.sync.dma_start(out=et_sb, in_=etd.ap())
        ones_sb = const.tile([128, 64], BF16)
        nc.sync.dma_start(out=ones_sb, in_=onesd.ap())

        slabs = ctx.enter_context(tc.tile_pool(name="slabs", bufs=2))
        yrow = ctx.enter_context(tc.tile_pool(name="yrow", bufs=3))
        work = ctx.enter_context(tc.tile_pool(name="work", bufs=3))
        pU = ctx.enter_context(tc.tile_pool(name="pU", bufs=1, space="PSUM"))
        pS = ctx.enter_context(tc.tile_pool(name="pS", bufs=1, space="PSUM"))
        pZ = ctx.enter_context(tc.tile_pool(name="pZ", bufs=2, space="PSUM"))
        pW = ctx.enter_context(tc.tile_pool(name="pW", bufs=2, space="PSUM"))
        pY = ctx.enter_context(tc.tile_pool(name="pY", bufs=2, space="PSUM"))

        for rep in range(repeat):
          for hp in range(4):
            cs = 128 * hp
            # whole-slab loads in rolled coords: slab[p, r', j'] =
            # src[c, (4+r')%128, (4+j')%128]
            x_slab = slabs.tile([128, 128, 128], F16, tag="xs")
            s_slab = slabs.tile([128, 128, 128], F16, tag="ss")
            for dst, src in ((x_slab, x_ap), (s_slab, s_ap)):
                sc = src[cs:cs + 128]
                nc.sync.dma_start(out=dst[:, 0:124, 0:124],
                                  in_=sc[:, 4:128, 4:128])
                nc.sync.dma_start(out=dst[:, 0:124, 124:128],
                                  in_=sc[:, 4:128, 0:4])
                nc.sync.dma_start(out=dst[:, 124:128, 0:124],
                                  in_=sc[:, 0:4, 4:128])
                nc.sync.dma_start(out=dst[:, 124:128, 124:128],
                                  in_=sc[:, 0:4, 0:4])
            for wi in range(NH):
                y_row = yrow.tile([128, 8, 128], F32, tag="yr")
                for half in range(2):
                    _supertile(nc, tc, dict(
                        x_row=x_slab[:, 8 * wi:8 * wi + 8, :],
                        s_row=s_slab[:, 8 * wi:8 * wi + 8, :],
                        g2=g2_sb, ht2=ht2_sb, et=et_sb, ones=ones_sb,
                        work=work, y_row=y_row,
                        pU=pU, pS=pS, pZ=pZ, pW=pW, pY=pY,
                        hp=hp, wi=wi, half=half))
                # store y_row with roll(+4,+4): row (8wi+4+ti)%128,
                # col j' -> (4+j')%128
                oc = o_ap[cs:cs + 128]
                if wi < 15:
                    rparts = [(0, 8, 8 * wi + 4)]
                else:
                    rparts = [(0, 4, 124), (4, 8, 0)]
                for (t0, t1, rd) in rparts:
                    nc.sync.dma_start(out=oc[:, rd:rd + t1 - t0, 4:128],
                                      in_=y_row[:, t0:t1, 0:124])
                    nc.sync.dma_start(out=oc[:, rd:rd + t1 - t0, 0:4],
                                      in_=y_row[:, t0:t1, 124:128])
    return nc


def _supertile(nc, tc, a):
    hp, wi, half = a["hp"], a["wi"], a["half"]
    x_row, s_row = a["x_row"], a["s_row"]
    work = a["work"]
    c0 = 64 * half
    var = (2 if wi == 15 else 0) + half

    # u' = G x  -> psum cols (si, j); copy to SBUF window-grouped (w, si, sj)
    u_ps = a["pU"].tile([128, 512], F32, tag="u")
    nc.tensor.matmul(out=u_ps, lhsT=a["g2"], rhs=x_row[:, :, c0:c0 + 64],
                     start=True, stop=True)
    u_sb = work.tile([128, 512], F16, tag="u_sb")
    nc.scalar.copy(
        out=u_sb.rearrange("p (w si sj) -> p w si sj", w=8, si=8, sj=8),
        in_=u_ps.rearrange("p (si w sj) -> p w si sj", si=8, w=8, sj=8))

    # window-grouped skip copy (w1 stationary operand needs contiguous free)
    skw = work.tile([128, 512], F16, tag="skw")
    nc.vector.tensor_copy(
        skw.rearrange("p (w ti tj) -> p w ti tj", w=8, ti=8, tj=8),
        s_row[:, :, c0:c0 + 64].rearrange(
            "p ti (w tj) -> p w ti tj", w=8, tj=8))

    # s^T blocks: [64h+ki, 64w+qi]
    s_ps = a["pS"].tile([128, 512], F32, tag="s")
    for h in range(2):
        hb = 64 * h
        for w in range(8):
            nc.tensor.matmul(
                out=s_ps[hb:hb + 64, 64 * w:64 * w + 64],
                lhsT=u_sb[hb:hb + 64, 64 * w:64 * w + 64],
                rhs=x_row[hb:hb + 64, :, c0 + 8 * w:c0 + 8 * w + 8],
                start=True, stop=True)

    # p_hat = exp(s) * ET   (bf16)
    p_raw = work.tile([128, 512], BF16, tag="p_raw")
    nc.scalar.activation(out=p_raw, in_=s_ps,
                         func=mybir.ActivationFunctionType.Exp)
    p_m = work.tile([128, 512], BF16, tag="p_m")
    nc.vector.tensor_mul(p_m, p_raw, a["et"][:, var, :])

    # Z broadcast to all partitions: ones[64,64]^T @ p_hat
    zb_ps = a["pZ"].tile([128, 512], F32, tag="zb")
    for h in range(2):
        hb = 64 * h
        nc.tensor.matmul(out=zb_ps[hb:hb + 64, :],
                         lhsT=a["ones"][hb:hb + 64, :],
                         rhs=p_m[hb:hb + 64, :], start=True, stop=True)
    rz = work.tile([128, 512], F32, tag="rz")
    nc.vector.reciprocal(rz, zb_ps)

    # w1 = (H skip_w)^T blocks [64h+ki, 64w+f]
    w1_ps = a["pW"].tile([128, 512], F32, tag="w1")
    for h in range(2):
        hb = 64 * h
        for w in range(8):
            nc.tensor.matmul(
                out=w1_ps[hb:hb + 64, 64 * w:64 * w + 64],
                lhsT=skw[hb:hb + 64, 64 * w:64 * w + 64],
                rhs=a["ht2"][hb:hb + 64, :], start=True, stop=True)
    w1_sb = work.tile([128, 512], BF16, tag="w1_sb")
    nc.vector.tensor_copy(w1_sb, w1_ps)

    # y blocks [64h+f, 64w+qi]
    y_ps = a["pY"].tile([128, 512], F32, tag="y")
    for h in range(2):
        hb = 64 * h
        for w in range(8):
            nc.tensor.matmul(
                out=y_ps[hb:hb + 64, 64 * w:64 * w + 64],
                lhsT=w1_sb[hb:hb + 64, 64 * w:64 * w + 64],
                rhs=p_m[hb:hb + 64, 64 * w:64 * w + 64],
                start=True, stop=True)
    # normalize + permute to image-row-major (ti, w, tj) into the wi-row tile
    nc.vector.tensor_mul(
        a["y_row"][:, :, c0:c0 + 64].rearrange(
            "p ti (w tj) -> p w ti tj", w=8, tj=8),
        y_ps.rearrange("p (w ti tj) -> p w ti tj", w=8, ti=8, tj=8),
        rz.rearrange("p (w ti tj) -> p w ti tj", w=8, ti=8, tj=8))


def _get_nc():
    global _CACHED_NC
    if _CACHED_NC is None:
        _CACHED_NC = _build_nc()
        _CACHED_NC.compile()
    return _CACHED_NC


# ---------------------------------------------------------------- entry point
def _run(skip, x, pe, Wq, Wk, Wv, Wl, **spmd_kwargs):
    import ml_dtypes
    skip = np.asarray(skip, np.float32)
    x = np.asarray(x, np.float32)
    bdg, ht2, et, ones = _make_consts(np.asarray(pe, np.float32),
                                      np.asarray(Wq, np.float32),
                                      np.asarray(Wk, np.float32),
                                      np.asarray(Wv, np.float32),
                                      np.asarray(Wl, np.float32))
    x16 = x.astype(np.float16)
    skip16 = skip.astype(np.float16)
    bdg16 = bdg.astype(np.float16)
    ht216 = ht2.astype(np.float16)
    etb = et.astype(ml_dtypes.bfloat16)
    onesb = ones.astype(ml_dtypes.bfloat16)

    nc = _get_nc()
    in_maps = [dict(x16=x16[b], skip16=skip16[b], g2=bdg16, ht2=ht216,
                    et=etb, ones1=onesb) for b in range(B)]
    res = run_bass_kernel_spmd(nc, in_maps, core_ids=list(range(8)),
                               **spmd_kwargs)
    return np.stack([r["out"] for r in res.results], axis=0), res


def kernel(skip, x, pe, Wq, Wk, Wv, Wl):
    out, _ = _run(skip, x, pe, Wq, Wk, Wv, Wl)
    return out
